# revision 8
# baseline (speedup 1.0000x reference)
"""Trainium2 Bass kernel for nn_DatTransformer (sparse hard-max attention).

Sharding: 8 cores = 4 batches x 2 query-halves. Each core holds full K for its
batch (keys in rolled query-half-first order).

Numerics v2: pure f32r (12-bit mantissa) matmuls everywhere -- no fp8
DoubleRow correction. Score error is bounded (~0.06 max on this data); rows
whose device top-2 margin is below TAU get recomputed exactly on the host
from the returned top-8 values/indices (~200 rows of 16384). The selection
threshold (0.95) is >100 away from every row max, so sel is always true and
is also verified host-side from the exported maxima.

  proj:   (wh f32r)^T (xh*2^13 f32r) -> psum = proj*2^13; Act w/ scale+bias
          writes f32r q*2^12 (Q) / k (K) directly.
  scores: (qh*2^12 f32r)^T (kh f32r) -> psum = score*2^12; Act copy w/ scale
          2^-12 -> SBUF scores; DVE max (top-8) + max_index -> winner+margin.
  out:    indirect-gather x[winner] (bf16) -> transpose -> @ (v_w.T@out_w.T)
          bf16 -> + (v_b@out_w.T + out_b) broadcast add (Pool).
"""
import sys, os

for _p in ("/root/.axon_site", "/root/.axon_site/_ro/trn_rl_repo",
           "/root/.axon_site/_ro/pypackages", "/opt/trn_rl_repo"):
    if os.path.isdir(_p) and _p not in sys.path:
        sys.path.append(_p)

import numpy as np
import concourse.bass as bass
import concourse.bacc as bacc
import concourse.mybir as mybir
from concourse.tile import TileContext
from concourse.bass_utils import run_bass_kernel_spmd
from concourse import masks

P = 128
S = 4096          # keys per batch
SQ = 2048         # queries per core
D = 512
NE = D // P       # 4 embedding chunks
NQT = SQ // P     # 16 query tiles
PC = 1024         # proj x-chunk width
TAU = 0.15        # host-fixup margin threshold (max observed dev err ~0.06)

F32 = mybir.dt.float32
F32R = mybir.dt.float32r
BF16 = mybir.dt.bfloat16
U32 = mybir.dt.uint32
AF = mybir.ActivationFunctionType
ALU = mybir.AluOpType

_CACHED = {}


def round_f32r(a: np.ndarray) -> np.ndarray:
    """Round fp32 array to the 12-explicit-mantissa-bit float32r grid (RNE)."""
    b = np.ascontiguousarray(a, dtype=np.float32).view(np.uint32)
    r = (b + 0x7FF + ((b >> 12) & 1)) & np.uint32(0xFFFFF000)
    return r.view(np.float32).copy()


def build_nc(repeat: int = 1, variant: str = "full"):
    """variant: 'full' | 'projonly' | 'nofin' (no finalize) | 'nodve'
    (no max/finalize) | 'scoreonly' (skip proj repeat; scores repeat)."""
    nc = bacc.Bacc("TRN2", target_bir_lowering=False, debug=False, num_devices=8)

    xh13_d = nc.declare_dram_parameter("xh13", [D, S], F32R, isOutput=False)
    qwh_d = nc.declare_dram_parameter("qwh", [D, D], F32R, isOutput=False)
    kwh_d = nc.declare_dram_parameter("kwh", [D, D], F32R, isOutput=False)
    q_bias = nc.declare_dram_parameter("q_bias", [D], F32, isOutput=False)
    k_bias = nc.declare_dram_parameter("k_bias", [D], F32, isOutput=False)
    xgbf_d = nc.declare_dram_parameter("xgbf", [S, D], BF16, isOutput=False)
    wvo_d = nc.declare_dram_parameter("wvo", [D, D], BF16, isOutput=False)
    bvo_d = nc.declare_dram_parameter("bvo_ob", [1, D], F32, isOutput=False)
    out_d = nc.declare_dram_parameter("out", [SQ, D], F32, isOutput=True)
    mx_d = nc.declare_dram_parameter("mx_out", [SQ, 1], F32, isOutput=True)
    cnt_d = nc.declare_dram_parameter("cnt_out", [SQ, 1], F32, isOutput=True)

    with TileContext(nc) as tc:
        from contextlib import nullcontext

        with tc.tile_pool(name="persist", bufs=1) as pp:
            qwh_t = [pp.tile([P, D], F32R, name=f"qwh{d}", tag=f"qwh{d}")
                     for d in range(NE)]
            kwh_t = [pp.tile([P, D], F32R, name=f"kwh{d}", tag=f"kwh{d}")
                     for d in range(NE)]
            wvo_t = [pp.tile([P, D], BF16, name=f"wvo{d}", tag=f"wvo{d}")
                     for d in range(NE)]
            qb_t = [pp.tile([P, 1], F32, name=f"qb{e}", tag=f"qb{e}")
                    for e in range(NE)]
            kb_t = [pp.tile([P, 1], F32, name=f"kb{e}", tag=f"kb{e}")
                    for e in range(NE)]
            for d in range(NE):
                rs = slice(d * P, (d + 1) * P)
                nc.sync.dma_start(out=qwh_t[d][:], in_=qwh_d[rs, :])
                nc.sync.dma_start(out=kwh_t[d][:], in_=kwh_d[rs, :])
                nc.sync.dma_start(out=wvo_t[d][:], in_=wvo_d[rs, :])
            q_bias_r = q_bias.rearrange("(e p) -> e p", p=P)
            k_bias_r = k_bias.rearrange("(e p) -> e p", p=P)
            for e in range(NE):
                nc.sync.dma_start(out=qb_t[e][:, 0], in_=q_bias_r[e])
                nc.sync.dma_start(out=kb_t[e][:, 0], in_=k_bias_r[e])
            ident = pp.tile([P, P], BF16, name="ident")
            masks.make_identity(nc, ident[:])
            bvo_bc = pp.tile([P, D], F32, name="bvo_bc")
            nc.sync.dma_start(out=bvo_bc[0:1, :], in_=bvo_d[:])
            nc.gpsimd.partition_broadcast(bvo_bc[:], bvo_bc[0:1, :])

            qt = [pp.tile([P, SQ], F32R, name=f"qt{e}", tag=f"qt{e}")
                  for e in range(NE)]
            kt = [pp.tile([P, S], F32R, name=f"kt{e}", tag=f"kt{e}")
                  for e in range(NE)]

            # ---------------- Phase A: projections (shared x loads) --------
            with tc.tile_pool(name="xc", bufs=2) as xcp, \
                 tc.tile_pool(name="pspj", bufs=2, space="PSUM") as pjp:

                def proj_chunk(c):
                    cs = slice(c * PC, (c + 1) * PC)
                    xh_c = [xcp.tile([P, PC], F32R, name=f"xh{d}", tag=f"xh{d}")
                            for d in range(NE)]
                    for d in range(NE):
                        rs = slice(d * P, (d + 1) * P)
                        nc.sync.dma_start(out=xh_c[d][:], in_=xh13_d[rs, cs])
                    projs = [(kwh_t, kb_t, kt, 2.0 ** -13)]
                    if c < SQ // PC:
                        projs.append((qwh_t, qb_t, qt, 2.0 ** -1))
                    for wt, bt, dest, scale in projs:
                        for e in range(NE):
                            es = slice(e * P, (e + 1) * P)
                            ps = pjp.tile([P, PC], F32, name="pspj", tag="pspj")
                            for d in range(NE):
                                for h in range(2):
                                    hs = slice(h * 512, (h + 1) * 512)
                                    nc.tensor.matmul(
                                        ps[:, hs], wt[d][:, es], xh_c[d][:, hs],
                                        start=(d == 0), stop=(d == NE - 1))
                            nc.scalar.activation(dest[e][:, cs], ps[:],
                                                 AF.Identity, bias=bt[e][:],
                                                 scale=scale)

                rq = repeat if variant in ("full", "projonly") else 1
                with (tc.For_i(0, rq, 1) if rq > 1 else nullcontext()):
                    for c in range(S // PC):
                        proj_chunk(c)

            # ---------------- Phase B: scores + argmax + output ------------
            with tc.tile_pool(name="scb", bufs=2) as scp, \
                 tc.tile_pool(name="st", bufs=2) as stp, \
                 tc.tile_pool(name="fin", bufs=2) as fp, \
                 tc.tile_pool(name="mm", bufs=2, space="PSUM") as mmp, \
                 tc.tile_pool(name="tp", bufs=2, space="PSUM") as tpp, \
                 tc.tile_pool(name="op", bufs=2, space="PSUM") as opp:

                rs_ = repeat if variant != "projonly" else 1
                with (tc.For_i(0, rs_, 1) if rs_ > 1 else nullcontext()):
                    for q in range(NQT):
                        qs = slice(q * P, (q + 1) * P)
                        sc_t = scp.tile([P, S], F32, name="sc", tag="sc")
                        for quarter in range(4):
                            ps = mmp.tile([P, 1024], F32, name="ps", tag="ps")
                            for e in range(NE):
                                for g in range(2):
                                    ks = slice(quarter * 1024 + g * 512,
                                               quarter * 1024 + (g + 1) * 512)
                                    nc.tensor.matmul(
                                        ps[:, g * 512:(g + 1) * 512],
                                        qt[e][:, qs], kt[e][:, ks],
                                        start=(e == 0), stop=(e == NE - 1))
                            nc.scalar.activation(
                                sc_t[:, quarter * 1024:(quarter + 1) * 1024],
                                ps[:], AF.Copy, scale=2.0 ** -12)
                        if variant == "nodve":
                            continue
                        gmax = stp.tile([P, 1], F32, name="gmax", tag="gmax")
                        nc.vector.tensor_reduce(gmax[:], sc_t[:], op=ALU.max,
                                                axis=mybir.AxisListType.X)
                        ix8 = stp.tile([P, 8], U32, name="ix8", tag="ix8")
                        nc.vector.max_index(
                            out=ix8[:],
                            in_max=gmax[:, 0:1].broadcast_to([P, 8]),
                            in_values=sc_t[:])
                        thr = stp.tile([P, 1], F32, name="thr", tag="thr")
                        nc.vector.tensor_scalar_add(thr[:], gmax[:],
                                                    -float(TAU))
                        msk = scp.tile([P, S], BF16, name="msk", tag="msk")
                        nc.vector.tensor_tensor(
                            msk[:], sc_t[:],
                            thr[:, 0:1].broadcast_to([P, S]), op=ALU.is_ge)
                        cnt = stp.tile([P, 1], F32, name="cnt", tag="cnt")
                        nc.vector.tensor_reduce(cnt[:], msk[:], op=ALU.add,
                                                axis=mybir.AxisListType.X)
                        nc.sync.dma_start(out=mx_d[qs, :], in_=gmax[:])
                        nc.sync.dma_start(out=cnt_d[qs, :], in_=cnt[:])
                        if variant == "nofin":
                            continue
                        # ---- finalize ----
                        xg = fp.tile([P, D], BF16, name="xg", tag="xg")
                        nc.gpsimd.indirect_dma_start(
                            out=xg[:], out_offset=None, in_=xgbf_d[:],
                            in_offset=bass.IndirectOffsetOnAxis(
                                ap=ix8[:, 0:1], axis=0))
                        pt = tpp.tile([P, D], BF16, name="pt", tag="pt")
                        for dch in range(NE):
                            nc.tensor.transpose(pt[:, dch * P:(dch + 1) * P],
                                                xg[:, dch * P:(dch + 1) * P],
                                                ident[:])
                        xgt = fp.tile([P, D], BF16, name="xgt", tag="xgt")
                        nc.scalar.activation(xgt[:], pt[:], AF.Copy)
                        po = opp.tile([P, D], F32, name="po", tag="po")
                        for dch in range(NE):
                            nc.tensor.matmul(po[:],
                                             xgt[:, dch * P:(dch + 1) * P],
                                             wvo_t[dch][:],
                                             start=(dch == 0),
                                             stop=(dch == NE - 1))
                        outt = fp.tile([P, D], F32, name="outt", tag="outt")
                        nc.vector.tensor_tensor(outt[:], po[:], bvo_bc[:],
                                                op=ALU.add)
                        nc.sync.dma_start(out=out_d[qs, :], in_=outt[:])

    nc.compile()
    return nc


def _get_nc(repeat: int = 1, variant: str = "full"):
    key = ("nc", repeat, variant)
    if key not in _CACHED:
        _CACHED[key] = build_nc(repeat, variant)
    return _CACHED[key]


def _prep_inputs(x, q_w, q_b, k_w, k_b, v_w, v_b, out_w, out_b):
    import ml_dtypes

    qwh = round_f32r(np.ascontiguousarray(q_w.T, dtype=np.float32))
    kwh = round_f32r(np.ascontiguousarray(k_w.T, dtype=np.float32))
    wvo = ((v_w.T.astype(np.float64) @ out_w.T.astype(np.float64))
           .astype(np.float32).astype(ml_dtypes.bfloat16))
    bvo_ob = (v_b.astype(np.float64) @ out_w.T.astype(np.float64)
              + out_b.astype(np.float64)).astype(np.float32)[None, :]

    in_maps = []
    for core in range(8):
        b, h = core // 2, core % 2
        xb = np.ascontiguousarray(x[:, b, :])                    # [S, D]
        order = np.r_[h * SQ:(h + 1) * SQ, (1 - h) * SQ:(2 - h) * SQ]
        xr = np.ascontiguousarray(xb[order])                     # rolled [S, D]
        xh13 = round_f32r(np.ascontiguousarray(xr.T)) * np.float32(2.0 ** 13)
        in_maps.append({
            "xh13": np.ascontiguousarray(xh13),
            "xgbf": np.ascontiguousarray(xr.astype(ml_dtypes.bfloat16)),
            "qwh": qwh, "kwh": kwh,
            # q_bias pre-scaled by 2^12: the Q-proj epilogue works on q*2^12
            "q_bias": np.ascontiguousarray(q_b * 4096.0, dtype=np.float32),
            "k_bias": np.ascontiguousarray(k_b, dtype=np.float32),
            "wvo": wvo, "bvo_ob": bvo_ob,
        })
    return in_maps


def _host_fixup(out, res, x, q_w, q_b, k_w, k_b, v_w, v_b, out_w, out_b):
    """Recompute rows whose device top-2 margin is < TAU (exact host math).
    Also covers threshold selection: rows with max < 2.0 get exact handling."""
    k_cache = {}

    def k_mat(b):
        if b not in k_cache:
            k_cache[b] = (x[:, b, :].astype(np.float64)
                          @ k_w.T.astype(np.float64) + k_b)
        return k_cache[b]

    n_patched = 0
    for core in range(8):
        b, h = core // 2, core % 2
        mx = res.results[core]["mx_out"][:, 0]    # [SQ] f32 row maxima
        cnt = res.results[core]["cnt_out"][:, 0]  # [SQ] #scores >= max-TAU
        risk = (cnt >= 1.5) | (mx < 2.0)
        rows = np.nonzero(risk)[0]
        if rows.size == 0:
            continue
        Kb = k_mat(b)                              # [S, D] f64, original order
        for r in rows:
            s = h * SQ + int(r)                    # original query index
            q_row = (x[s, b].astype(np.float64)
                     @ q_w.T.astype(np.float64) + q_b)
            sc = Kb @ q_row
            jmax = int(sc.argmax())
            if sc[jmax] >= 0.95:
                v_row = (x[jmax, b].astype(np.float64)
                         @ v_w.T.astype(np.float64) + v_b)
            else:
                v_row = np.zeros(D, dtype=np.float64)
            out[s, b, :] = (v_row @ out_w.T.astype(np.float64)
                            + out_b).astype(np.float32)
            n_patched += 1
    return n_patched


def kernel(x, q_w, q_b, k_w, k_b, v_w, v_b, out_w, out_b, _trace=False,
           **trace_kwargs):
    # accept jax or numpy inputs
    x, q_w, q_b, k_w, k_b, v_w, v_b, out_w, out_b = (
        np.asarray(a, dtype=np.float32)
        for a in (x, q_w, q_b, k_w, k_b, v_w, v_b, out_w, out_b))
    nc = _get_nc()
    in_maps = _prep_inputs(x, q_w, q_b, k_w, k_b, v_w, v_b, out_w, out_b)
    res = run_bass_kernel_spmd(nc, in_maps, list(range(8)), trace=_trace,
                               **trace_kwargs)
    out = np.empty((S, 4, D), dtype=np.float32)
    for core in range(8):
        b, h = core // 2, core % 2
        out[h * SQ:(h + 1) * SQ, b, :] = res.results[core]["out"]
    _host_fixup(out, res, x, q_w, q_b, k_w, k_b, v_w, v_b, out_w, out_b)
    if _trace:
        _CACHED["last_results"] = res
    return out


# revision 11
# speedup vs baseline: 1.2450x; 1.2450x over previous
"""Trainium2 Bass kernel for nn_DatTransformer (sparse hard-max attention).

Sharding: 8 cores = 4 batches x 2 query-halves. Each core holds full K for its
batch (keys in rolled query-half-first order).

Numerics v2: pure f32r (12-bit mantissa) matmuls everywhere -- no fp8
DoubleRow correction. Score error is bounded (~0.06 max on this data); rows
whose device top-2 margin is below TAU get recomputed exactly on the host
from the returned top-8 values/indices (~200 rows of 16384). The selection
threshold (0.95) is >100 away from every row max, so sel is always true and
is also verified host-side from the exported maxima.

  proj:   (wh f32r)^T (xh*2^13 f32r) -> psum = proj*2^13; Act w/ scale+bias
          writes f32r q*2^12 (Q) / k (K) directly.
  scores: (qh*2^12 f32r)^T (kh f32r) -> psum = score*2^12; Act copy w/ scale
          2^-12 -> SBUF scores; DVE max (top-8) + max_index -> winner+margin.
  out:    indirect-gather x[winner] (bf16) -> transpose -> @ (v_w.T@out_w.T)
          bf16 -> + (v_b@out_w.T + out_b) broadcast add (Pool).
"""
import sys, os

for _p in ("/root/.axon_site", "/root/.axon_site/_ro/trn_rl_repo",
           "/root/.axon_site/_ro/pypackages", "/opt/trn_rl_repo"):
    if os.path.isdir(_p) and _p not in sys.path:
        sys.path.append(_p)

import numpy as np
import concourse.bass as bass
import concourse.bacc as bacc
import concourse.mybir as mybir
from concourse.tile import TileContext
from concourse.bass_utils import run_bass_kernel_spmd
from concourse import masks

P = 128
S = 4096          # keys per batch
SQ = 2048         # queries per core
D = 512
NE = D // P       # 4 embedding chunks
NQT = SQ // P     # 16 query tiles
PC = 1024         # proj x-chunk width
TAU = 0.15        # host-fixup margin threshold (max observed dev err ~0.06)

F32 = mybir.dt.float32
F32R = mybir.dt.float32r
BF16 = mybir.dt.bfloat16
U32 = mybir.dt.uint32
AF = mybir.ActivationFunctionType
ALU = mybir.AluOpType

_CACHED = {}


def round_f32r(a: np.ndarray) -> np.ndarray:
    """Round fp32 array to the 12-explicit-mantissa-bit float32r grid (RNE)."""
    b = np.ascontiguousarray(a, dtype=np.float32).view(np.uint32)
    r = (b + 0x7FF + ((b >> 12) & 1)) & np.uint32(0xFFFFF000)
    return r.view(np.float32).copy()


def build_nc(repeat: int = 1, variant: str = "full"):
    """variant: 'full' | 'projonly' | 'nofin' (no finalize) | 'nodve'
    (no max/finalize) | 'scoreonly' (skip proj repeat; scores repeat)."""
    nc = bacc.Bacc("TRN2", target_bir_lowering=False, debug=False, num_devices=8)

    xh13_d = nc.declare_dram_parameter("xh13", [D, S], F32R, isOutput=False)
    qwh_d = nc.declare_dram_parameter("qwh", [D, D], F32R, isOutput=False)
    kwh_d = nc.declare_dram_parameter("kwh", [D, D], F32R, isOutput=False)
    q_bias = nc.declare_dram_parameter("q_bias", [D], F32, isOutput=False)
    k_bias = nc.declare_dram_parameter("k_bias", [D], F32, isOutput=False)
    xgbf_d = nc.declare_dram_parameter("xgbf", [S, D], BF16, isOutput=False)
    wvo_d = nc.declare_dram_parameter("wvo", [D, D], BF16, isOutput=False)
    bvo_d = nc.declare_dram_parameter("bvo_ob", [1, D], F32, isOutput=False)
    out_d = nc.declare_dram_parameter("out", [SQ, D], F32, isOutput=True)
    mx_d = nc.declare_dram_parameter("mx_out", [SQ, 1], F32, isOutput=True)
    cnt_d = nc.declare_dram_parameter("cnt_out", [SQ, 1], F32, isOutput=True)

    with TileContext(nc) as tc:
        from contextlib import nullcontext

        with tc.tile_pool(name="persist", bufs=1) as pp:
            qwh_t = [pp.tile([P, D], F32R, name=f"qwh{d}", tag=f"qwh{d}")
                     for d in range(NE)]
            kwh_t = [pp.tile([P, D], F32R, name=f"kwh{d}", tag=f"kwh{d}")
                     for d in range(NE)]
            wvo_t = [pp.tile([P, D], BF16, name=f"wvo{d}", tag=f"wvo{d}")
                     for d in range(NE)]
            qb_t = [pp.tile([P, 1], F32, name=f"qb{e}", tag=f"qb{e}")
                    for e in range(NE)]
            kb_t = [pp.tile([P, 1], F32, name=f"kb{e}", tag=f"kb{e}")
                    for e in range(NE)]
            for d in range(NE):
                rs = slice(d * P, (d + 1) * P)
                nc.sync.dma_start(out=qwh_t[d][:], in_=qwh_d[rs, :])
                nc.sync.dma_start(out=kwh_t[d][:], in_=kwh_d[rs, :])
                nc.sync.dma_start(out=wvo_t[d][:], in_=wvo_d[rs, :])
            q_bias_r = q_bias.rearrange("(e p) -> e p", p=P)
            k_bias_r = k_bias.rearrange("(e p) -> e p", p=P)
            for e in range(NE):
                nc.sync.dma_start(out=qb_t[e][:, 0], in_=q_bias_r[e])
                nc.sync.dma_start(out=kb_t[e][:, 0], in_=k_bias_r[e])
            ident = pp.tile([P, P], BF16, name="ident")
            masks.make_identity(nc, ident[:])
            bvo_bc = pp.tile([P, D], F32, name="bvo_bc")
            nc.sync.dma_start(out=bvo_bc[0:1, :], in_=bvo_d[:])
            nc.gpsimd.partition_broadcast(bvo_bc[:], bvo_bc[0:1, :])

            qt = [pp.tile([P, SQ], F32R, name=f"qt{e}", tag=f"qt{e}")
                  for e in range(NE)]
            kt = [pp.tile([P, S], F32R, name=f"kt{e}", tag=f"kt{e}")
                  for e in range(NE)]

            # ---------------- Phase A: projections (shared x loads) --------
            with tc.tile_pool(name="xc", bufs=2) as xcp, \
                 tc.tile_pool(name="pspj", bufs=2, space="PSUM") as pjp:

                def proj_chunk(c):
                    cs = slice(c * PC, (c + 1) * PC)
                    xh_c = [xcp.tile([P, PC], F32R, name=f"xh{d}", tag=f"xh{d}")
                            for d in range(NE)]
                    for d in range(NE):
                        rs = slice(d * P, (d + 1) * P)
                        nc.sync.dma_start(out=xh_c[d][:], in_=xh13_d[rs, cs])
                    projs = [(kwh_t, kb_t, kt, 2.0 ** -13)]
                    if c < SQ // PC:
                        projs.append((qwh_t, qb_t, qt, 2.0 ** -1))
                    for wt, bt, dest, scale in projs:
                        for e in range(NE):
                            es = slice(e * P, (e + 1) * P)
                            ps = pjp.tile([P, PC], F32, name="pspj", tag="pspj")
                            for d in range(NE):
                                for h in range(2):
                                    hs = slice(h * 512, (h + 1) * 512)
                                    nc.tensor.matmul(
                                        ps[:, hs], wt[d][:, es], xh_c[d][:, hs],
                                        start=(d == 0), stop=(d == NE - 1))
                            nc.scalar.activation(dest[e][:, cs], ps[:],
                                                 AF.Identity, bias=bt[e][:],
                                                 scale=scale)

                rq = repeat if variant in ("full", "projonly") else 1
                with (tc.For_i(0, rq, 1) if rq > 1 else nullcontext()):
                    for c in range(S // PC):
                        proj_chunk(c)

            # ---------------- Phase B: scores + argmax + output ------------
            with tc.tile_pool(name="scb", bufs=2) as scp, \
                 tc.tile_pool(name="st", bufs=2) as stp, \
                 tc.tile_pool(name="fin", bufs=2) as fp, \
                 tc.tile_pool(name="mm", bufs=2, space="PSUM") as mmp, \
                 tc.tile_pool(name="tp", bufs=2, space="PSUM") as tpp, \
                 tc.tile_pool(name="op", bufs=2, space="PSUM") as opp:

                rs_ = repeat if variant != "projonly" else 1
                with (tc.For_i(0, rs_, 1) if rs_ > 1 else nullcontext()):
                    for q in range(NQT):
                        qs = slice(q * P, (q + 1) * P)
                        sc_t = scp.tile([P, S], F32, name="sc", tag="sc")
                        for quarter in range(4):
                            ps = mmp.tile([P, 1024], F32, name="ps", tag="ps")
                            for e in range(NE):
                                for g in range(2):
                                    ks = slice(quarter * 1024 + g * 512,
                                               quarter * 1024 + (g + 1) * 512)
                                    nc.tensor.matmul(
                                        ps[:, g * 512:(g + 1) * 512],
                                        qt[e][:, qs], kt[e][:, ks],
                                        start=(e == 0), stop=(e == NE - 1))
                            nc.scalar.activation(
                                sc_t[:, quarter * 1024:(quarter + 1) * 1024],
                                ps[:], AF.Copy, scale=2.0 ** -12)
                        if variant == "nodve":
                            continue
                        gmax = stp.tile([P, 1], F32, name="gmax", tag="gmax")
                        nc.vector.tensor_reduce(gmax[:], sc_t[:], op=ALU.max,
                                                axis=mybir.AxisListType.X)
                        ix8 = stp.tile([P, 8], U32, name="ix8", tag="ix8")
                        nc.vector.max_index(
                            out=ix8[:],
                            in_max=gmax[:, 0:1].broadcast_to([P, 8]),
                            in_values=sc_t[:])
                        # nthr = TAU - gmax; Sign(sc + nthr) summed on the Act
                        # engine gives (#above - #below) vs thresh gmax-TAU.
                        nthr = stp.tile([P, 1], F32, name="nthr", tag="nthr")
                        nc.vector.tensor_scalar(nthr[:], gmax[:], -1.0,
                                                float(TAU), op0=ALU.mult,
                                                op1=ALU.add)
                        sgn = scp.tile([P, S], BF16, name="sgn", tag="sgn")
                        cnt = stp.tile([P, 1], F32, name="cnt", tag="cnt")
                        nc.scalar.activation(sgn[:], sc_t[:], AF.Sign,
                                             bias=nthr[:, 0:1],
                                             accum_out=cnt[:])
                        nc.sync.dma_start(out=mx_d[qs, :], in_=gmax[:])
                        nc.sync.dma_start(out=cnt_d[qs, :], in_=cnt[:])
                        if variant == "nofin":
                            continue
                        # ---- finalize ----
                        xg = fp.tile([P, D], BF16, name="xg", tag="xg")
                        nc.gpsimd.indirect_dma_start(
                            out=xg[:], out_offset=None, in_=xgbf_d[:],
                            in_offset=bass.IndirectOffsetOnAxis(
                                ap=ix8[:, 0:1], axis=0))
                        pt = tpp.tile([P, D], BF16, name="pt", tag="pt")
                        for dch in range(NE):
                            nc.tensor.transpose(pt[:, dch * P:(dch + 1) * P],
                                                xg[:, dch * P:(dch + 1) * P],
                                                ident[:])
                        xgt = fp.tile([P, D], BF16, name="xgt", tag="xgt")
                        nc.scalar.activation(xgt[:], pt[:], AF.Copy)
                        po = opp.tile([P, D], F32, name="po", tag="po")
                        for dch in range(NE):
                            nc.tensor.matmul(po[:],
                                             xgt[:, dch * P:(dch + 1) * P],
                                             wvo_t[dch][:],
                                             start=(dch == 0),
                                             stop=(dch == NE - 1))
                        outt = fp.tile([P, D], F32, name="outt", tag="outt")
                        nc.vector.tensor_tensor(outt[:], po[:], bvo_bc[:],
                                                op=ALU.add)
                        nc.sync.dma_start(out=out_d[qs, :], in_=outt[:])

    nc.compile()
    return nc


def _get_nc(repeat: int = 1, variant: str = "full"):
    key = ("nc", repeat, variant)
    if key not in _CACHED:
        _CACHED[key] = build_nc(repeat, variant)
    return _CACHED[key]


def _prep_inputs(x, q_w, q_b, k_w, k_b, v_w, v_b, out_w, out_b):
    import ml_dtypes

    qwh = round_f32r(np.ascontiguousarray(q_w.T, dtype=np.float32))
    kwh = round_f32r(np.ascontiguousarray(k_w.T, dtype=np.float32))
    wvo = ((v_w.T.astype(np.float64) @ out_w.T.astype(np.float64))
           .astype(np.float32).astype(ml_dtypes.bfloat16))
    bvo_ob = (v_b.astype(np.float64) @ out_w.T.astype(np.float64)
              + out_b.astype(np.float64)).astype(np.float32)[None, :]

    in_maps = []
    for core in range(8):
        b, h = core // 2, core % 2
        xb = np.ascontiguousarray(x[:, b, :])                    # [S, D]
        order = np.r_[h * SQ:(h + 1) * SQ, (1 - h) * SQ:(2 - h) * SQ]
        xr = np.ascontiguousarray(xb[order])                     # rolled [S, D]
        xh13 = round_f32r(np.ascontiguousarray(xr.T)) * np.float32(2.0 ** 13)
        in_maps.append({
            "xh13": np.ascontiguousarray(xh13),
            "xgbf": np.ascontiguousarray(xr.astype(ml_dtypes.bfloat16)),
            "qwh": qwh, "kwh": kwh,
            # q_bias pre-scaled by 2^12: the Q-proj epilogue works on q*2^12
            "q_bias": np.ascontiguousarray(q_b * 4096.0, dtype=np.float32),
            "k_bias": np.ascontiguousarray(k_b, dtype=np.float32),
            "wvo": wvo, "bvo_ob": bvo_ob,
        })
    return in_maps


def _host_fixup(out, res, x, q_w, q_b, k_w, k_b, v_w, v_b, out_w, out_b):
    """Recompute rows whose device top-2 margin is < TAU (exact host math).
    Also covers threshold selection: rows with max < 2.0 get exact handling."""
    k_cache = {}

    def k_mat(b):
        if b not in k_cache:
            k_cache[b] = (x[:, b, :].astype(np.float64)
                          @ k_w.T.astype(np.float64) + k_b)
        return k_cache[b]

    n_patched = 0
    for core in range(8):
        b, h = core // 2, core % 2
        mx = res.results[core]["mx_out"][:, 0]    # [SQ] f32 row maxima
        cnt = res.results[core]["cnt_out"][:, 0]  # [SQ] #above - #below
        # exactly one score (the max) within TAU of the max -> cnt <= -(S-2)
        risk = (cnt >= -(S - 2.5)) | (mx < 2.0)
        rows = np.nonzero(risk)[0]
        if rows.size == 0:
            continue
        Kb = k_mat(b)                              # [S, D] f64, original order
        for r in rows:
            s = h * SQ + int(r)                    # original query index
            q_row = (x[s, b].astype(np.float64)
                     @ q_w.T.astype(np.float64) + q_b)
            sc = Kb @ q_row
            jmax = int(sc.argmax())
            if sc[jmax] >= 0.95:
                v_row = (x[jmax, b].astype(np.float64)
                         @ v_w.T.astype(np.float64) + v_b)
            else:
                v_row = np.zeros(D, dtype=np.float64)
            out[s, b, :] = (v_row @ out_w.T.astype(np.float64)
                            + out_b).astype(np.float32)
            n_patched += 1
    return n_patched


def kernel(x, q_w, q_b, k_w, k_b, v_w, v_b, out_w, out_b, _trace=False,
           **trace_kwargs):
    # accept jax or numpy inputs
    x, q_w, q_b, k_w, k_b, v_w, v_b, out_w, out_b = (
        np.asarray(a, dtype=np.float32)
        for a in (x, q_w, q_b, k_w, k_b, v_w, v_b, out_w, out_b))
    nc = _get_nc()
    in_maps = _prep_inputs(x, q_w, q_b, k_w, k_b, v_w, v_b, out_w, out_b)
    res = run_bass_kernel_spmd(nc, in_maps, list(range(8)), trace=_trace,
                               **trace_kwargs)
    out = np.empty((S, 4, D), dtype=np.float32)
    for core in range(8):
        b, h = core // 2, core % 2
        out[h * SQ:(h + 1) * SQ, b, :] = res.results[core]["out"]
    _host_fixup(out, res, x, q_w, q_b, k_w, k_b, v_w, v_b, out_w, out_b)
    if _trace:
        _CACHED["last_results"] = res
    return out
